# revision 14
# baseline (speedup 1.0000x reference)
"""Trainium2 Bass kernel for nn_Conv2dTB (BN -> ternary quantize -> 3x3 conv
-> beta box-filter scaling), data-parallel over batch on 8 NeuronCores.

Contract: kernel(**inputs) takes the FULL unsharded inputs as numpy arrays and
returns the FULL [16, 256, 56, 56] float32 output. Internally the batch dim is
split 2 images/core; BN batch statistics use an on-device AllReduce so the
normalization matches the reference's full-batch statistics.

Phase structure (per core):
  1. dummy 4-byte AllReduce at t=0 absorbs inter-core launch skew so the real
     stats AllReduce later sees aligned peers.
  2. x streams in as 16 quarter-chunks on HWDGE; per-chunk channel sums run on
     GpSimd (reduce_sum) and channel sum-of-squares on Scalar (Square+accum),
     so stats lag the DMA by ~1us.
  3. stats AllReduce; its latency window is filled with weight bf16 conversion
     (GpSimd), PE transposes of the conv weights, and the beta-denominator
     box filter of a ones-grid (DVE).
  4. post-AR: ternarize (Scalar Sign) into flat zero-padded [58x58+2] grids,
     then the conv runs as fully contiguous 464-wide rhs slices (junk columns
     at the row seams are dropped by the output read view).
  5. beta path: channel sums via ones-stationary matmuls (output is broadcast
     across partitions), 3x3 box filter as +-1/+-58 flat shifted adds on DVE.
"""

import numpy as np

# Problem shapes (hardcoded per contract).
N, C, H, W = 16, 256, 56, 56
COUT = 256
KS = 3
EPS = 1e-4
N_CORES = 8
NLOC = N // N_CORES  # images per core (2)
CB = C // 128  # channel blocks (2)
COB = COUT // 128  # cout blocks (2)
RT_ROWS = 8  # image rows per pixel tile
NT = H // RT_ROWS  # row tiles per image (7)
NPIX = RT_ROWS * W  # valid pixels per tile (448)
HW = H * W  # 3136
PH = H + 2  # padded rows (58)
PW = W + 2  # padded cols (58)
GFLAT = PH * PW + 2  # flat padded grid: 1 front pad + 58*58 + 1 tail pad
FD_CONV = RT_ROWS * PW  # matmul free size incl junk cols (464)
QUARTER = HW // 4  # stats chunk (784)
COUNT = float(N * H * W)  # BN reduction count (full batch)

_CACHE = {}


def _build():
    import concourse.tile as tile
    import concourse.bass as bass
    from concourse import bacc, mybir

    f32 = mybir.dt.float32
    bf16 = mybir.dt.bfloat16
    AF = mybir.ActivationFunctionType
    ALU = mybir.AluOpType
    AX = mybir.AxisListType

    nc = bacc.Bacc("TRN2", target_bir_lowering=False, debug=False,
                   num_devices=N_CORES)

    # ---- external I/O ----
    x_d = nc.dram_tensor("x", [NLOC, C, H, W], f32, kind="ExternalInput").ap()
    gamma_d = nc.dram_tensor("bn_gamma", [C], f32, kind="ExternalInput").ap()
    bnbeta_d = nc.dram_tensor("bn_beta", [C], f32, kind="ExternalInput").ap()
    w_d = nc.dram_tensor("conv_w", [COUT, C, KS, KS], f32,
                         kind="ExternalInput").ap()
    cb_d = nc.dram_tensor("conv_b", [COUT], f32, kind="ExternalInput").ap()
    bb_d = nc.dram_tensor("beta_conv_b", [1], f32, kind="ExternalInput").ap()
    ident_d = nc.dram_tensor("ident128", [128, 128], f32,
                             kind="ExternalInput").ap()
    out_d = nc.dram_tensor("out", [NLOC, COUT, H, W], f32,
                           kind="ExternalOutput").ap()

    def gview(t, off, dims):
        """Strided view into a flat [128, GFLAT] tile at element offset."""
        a = t[:, 0:1]
        return bass.AP(tensor=a.tensor, offset=a.offset + off,
                       ap=[list(a.ap[0])] + [list(d) for d in dims])

    with tile.TileContext(nc) as tc:
        with (
            nc.allow_low_precision(
                reason="bf16 box-filter/beta path; error budget 2e-2"),
            tc.tile_pool(name="persist", bufs=1) as persist,
            tc.tile_pool(name="stage", bufs=4) as stage,
            tc.tile_pool(name="sq", bufs=2) as sqp,
            tc.tile_pool(name="denp", bufs=1) as denp,
            tc.tile_pool(name="xc", bufs=1) as xcp,
            tc.tile_pool(name="grid", bufs=2) as gridp,
            tc.tile_pool(name="boxt", bufs=1) as boxp,
            tc.tile_pool(name="outp", bufs=4) as outp,
            tc.tile_pool(name="ps_y", bufs=2, space="PSUM") as ps_y,
            tc.tile_pool(name="ps_c", bufs=2, space="PSUM") as ps_c,
            tc.tile_pool(name="ps_m", bufs=2, space="PSUM") as ps_m,
            tc.tile_pool(name="dram", bufs=1, space="DRAM") as dram,
        ):
            # ---------------- 0. dummy AllReduce: absorb launch skew --------
            dummy_in = dram.tile([1, 1], f32, name="dummy_in")
            dummy_out = dram.tile([1, 1], f32, name="dummy_out")
            nc.gpsimd.collective_compute(
                "AllReduce", ALU.add,
                replica_groups=[list(range(N_CORES))],
                ins=[dummy_in.opt()], outs=[dummy_out.opt()],
            )

            # ---------------- 1. tiny consts (sync queue, before x) ---------
            gamma_sb = persist.tile([128, CB], f32)
            nc.sync.dma_start(out=gamma_sb[:],
                              in_=gamma_d.rearrange("(cb p) -> p cb", p=128))
            bnbeta_sb = persist.tile([128, CB], f32)
            nc.sync.dma_start(out=bnbeta_sb[:],
                              in_=bnbeta_d.rearrange("(cb p) -> p cb", p=128))
            ident_sb = persist.tile([128, 128], f32)
            nc.sync.dma_start(out=ident_sb[:], in_=ident_d[:])

            # ---------------- 2. weight load on SWDGE (parallel queue) ------
            w_sb = persist.tile([128, COB, C, KS * KS], f32)
            wv = w_d.rearrange("(cob p) c k1 k2 -> cob p c (k1 k2)", p=128)
            for cob in range(COB):
                for a in range(2):
                    sl = slice(a * (C // 2), (a + 1) * (C // 2))
                    nc.gpsimd.dma_start(out=w_sb[:, cob, sl, :],
                                        in_=wv[cob][:, sl, :])

            # ---------------- 3. x stream + stats (sum on GpSimd, sumsq on
            # Scalar), chunked so stats trail the DMA by ~1 chunk ------------
            x_sb = persist.tile([128, NLOC, CB, HW], f32)
            gsum = persist.tile([128, CB, NLOC * 4], f32)
            ssq = persist.tile([128, CB, NLOC * 4], f32)
            for img in range(NLOC):
                xv = x_d[img].rearrange("(cb p) h w -> cb p (h w)", p=128)
                for cbk in range(CB):
                    for q in range(4):
                        sl = slice(q * QUARTER, (q + 1) * QUARTER)
                        nc.sync.dma_start(out=x_sb[:, img, cbk, sl],
                                          in_=xv[cbk][:, sl])
                        k = img * 4 + q
                        nc.vector.reduce_sum(gsum[:, cbk, k:k + 1],
                                             x_sb[:, img, cbk, sl], axis=AX.X)
                        sq = sqp.tile([128, QUARTER], bf16, tag="sq")
                        nc.scalar.activation(sq[:], x_sb[:, img, cbk, sl],
                                             AF.Square,
                                             accum_out=ssq[:, cbk, k:k + 1])

            # ---------------- 4. combine + AllReduce ------------------------
            partial = persist.tile([128, 2 * CB], f32)
            for cbk in range(CB):
                nc.vector.reduce_sum(partial[:, 2 * cbk:2 * cbk + 1],
                                     gsum[:, cbk, :], axis=AX.X)
                nc.vector.reduce_sum(partial[:, 2 * cbk + 1:2 * cbk + 2],
                                     ssq[:, cbk, :], axis=AX.X)
            bounce_in = dram.tile([128, 2 * CB], f32)
            bounce_out = dram.tile([128, 2 * CB], f32)
            nc.sync.dma_start(out=bounce_in[:], in_=partial[:])
            nc.gpsimd.collective_compute(
                "AllReduce", ALU.add,
                replica_groups=[list(range(N_CORES))],
                ins=[bounce_in.opt()], outs=[bounce_out.opt()],
            )
            allred = persist.tile([128, 2 * CB], f32)
            nc.sync.dma_start(out=allred[:], in_=bounce_out[:])

            # ---------------- 5. late consts --------------------------------
            convb_cols = persist.tile([128, COB], f32)
            nc.sync.dma_start(out=convb_cols[:],
                              in_=cb_d.rearrange("(cob p) -> p cob", p=128))
            bb_bc = persist.tile([128, 1], f32)
            bbsrc = bb_d[0:1]
            nc.sync.dma_start(
                out=bb_bc[:],
                in_=bass.AP(tensor=bbsrc.tensor, offset=bbsrc.offset,
                            ap=[[0, 128], [1, 1]]),
            )

            # ---------------- 6. DVE prep: ident bf16, t grids, beta denom --
            ident_bf = persist.tile([128, 128], bf16)
            nc.vector.tensor_copy(ident_bf[:], ident_sb[:])
            ones_st = persist.tile([128, 128], bf16)
            nc.vector.memset(ones_st[:], 1.0)

            # ternary grids: flat [1 pad][58x58][1 pad], zeroed
            t_t = [[persist.tile([128, GFLAT], bf16, name=f"t{cbk}{img}")
                    for img in range(NLOC)] for cbk in range(CB)]
            for cbk in range(CB):
                for img in range(NLOC):
                    nc.vector.memset(t_t[cbk][img][:], 0.0)

            def box(g, rpool):
                # horizontal 3-tap into r2, then vertical 3-tap written back
                # in place over g (g is dead once r2 is computed)
                r2 = rpool.tile([128, GFLAT], bf16, tag="r2")
                nc.vector.tensor_add(r2[:, 1:GFLAT - 1], g[:, 0:GFLAT - 2],
                                     g[:, 1:GFLAT - 1])
                nc.vector.tensor_add(r2[:, 1:GFLAT - 1], r2[:, 1:GFLAT - 1],
                                     g[:, 2:GFLAT])
                lo = 1 + PW
                hi = 1 + (PH - 1) * PW
                nc.vector.tensor_add(g[:, lo:hi], r2[:, lo - PW:hi - PW],
                                     r2[:, lo:hi])
                nc.vector.tensor_add(g[:, lo:hi], g[:, lo:hi],
                                     r2[:, lo + PW:hi + PW])
                return g

            # ---------------- 7. weight prep (bf16 + PE transpose) ----------
            w_bf = persist.tile([128, COB, C, KS * KS], bf16)
            for cob in range(COB):
                nc.gpsimd.tensor_copy(w_bf[:, cob, :, :], w_sb[:, cob, :, :])
            wT = persist.tile([128, CB, KS * KS, COB, 128], bf16)
            for cob in range(COB):
                for cbk in range(CB):
                    for tap in range(KS * KS):
                        wsl = w_bf[:, cob, cbk * 128:(cbk + 1) * 128, tap]
                        ps_t = ps_m.tile([128, 128], bf16, tag="psm")
                        nc.tensor.transpose(ps_t[:], wsl, ident_bf[:])
                        nc.scalar.copy(wT[:, cbk, tap, cob, :], ps_t[:])

            # ---------------- 8. scale/shift from AllReduce result ----------
            scale = persist.tile([128, CB], f32)
            shift = persist.tile([128, CB], f32)
            for cbk in range(CB):
                mean = stage.tile([128, 1], f32, tag="mean")
                nc.vector.tensor_scalar_mul(mean[:],
                                            allred[:, 2 * cbk:2 * cbk + 1],
                                            1.0 / COUNT)
                var = stage.tile([128, 1], f32, tag="var")
                nc.vector.tensor_mul(var[:], mean[:], mean[:])
                ex2 = stage.tile([128, 1], f32, tag="ex2")
                nc.vector.tensor_scalar_mul(
                    ex2[:], allred[:, 2 * cbk + 1:2 * cbk + 2], 1.0 / COUNT)
                nc.vector.tensor_sub(var[:], ex2[:], var[:])
                nc.vector.tensor_scalar_add(var[:], var[:], EPS)
                rvar = stage.tile([128, 1], f32, tag="rvar")
                nc.vector.reciprocal(rvar[:], var[:])
                rstd = stage.tile([128, 1], f32, tag="rstd")
                nc.scalar.sqrt(rstd[:], rvar[:])
                nc.vector.tensor_mul(scale[:, cbk:cbk + 1], rstd[:],
                                     gamma_sb[:, cbk:cbk + 1])
                nc.vector.tensor_mul(shift[:, cbk:cbk + 1], mean[:],
                                     scale[:, cbk:cbk + 1])
                nc.vector.tensor_sub(shift[:, cbk:cbk + 1],
                                     bnbeta_sb[:, cbk:cbk + 1],
                                     shift[:, cbk:cbk + 1])

            # ---------------- 9. ternarize (signs gate the conv) ------------
            for img in range(NLOC):
                for cbk in range(CB):
                    tv = gview(t_t[cbk][img], 1 + PW + 1, [[PW, H], [1, W]])
                    nc.scalar.activation(
                        tv, x_sb[:, img, cbk, :].rearrange(
                            "p (h w) -> p h w", w=W),
                        AF.Sign, bias=shift[:, cbk:cbk + 1],
                        scale=scale[:, cbk:cbk + 1])

            # beta denominator: box-filter a ones-grid, den=256*box+bb,
            # invden = 1/den (unpadded [128, HW] layout)
            g1 = gridp.tile([128, GFLAT], bf16, tag="g")
            nc.vector.memset(g1[:], 1.0)
            nc.vector.memset(g1[:, 0:1], 0.0)
            nc.vector.memset(g1[:, GFLAT - 1:GFLAT], 0.0)
            nc.vector.memset(g1[:, 1:1 + PW], 0.0)  # top pad row
            nc.vector.memset(g1[:, 1 + (PH - 1) * PW:1 + PH * PW], 0.0)
            nc.vector.memset(gview(g1, 1 + PW, [[PW, PH - 2], [1, 1]]), 0.0)
            nc.vector.memset(gview(g1, 1 + PW + PW - 1,
                                   [[PW, PH - 2], [1, 1]]), 0.0)
            b_ones = box(g1, boxp)
            den = denp.tile([128, HW], bf16, tag="den")
            nc.vector.tensor_scalar(
                den[:], gview(b_ones, 1 + PW + 1, [[PW, H], [1, W]]),
                256.0, bb_bc[:], ALU.mult, ALU.add)
            invden = persist.tile([128, HW], bf16)
            nc.vector.reciprocal(invden[:], den[:])

            # ---------------- 10. beta path per image -----------------------
            bbc = persist.tile([128, NLOC, HW], bf16)
            for img in range(NLOC):
                xcs = []
                for cbk in range(CB):
                    xc = xcp.tile([128, HW], bf16, tag=f"xc{cbk}")
                    nc.scalar.activation(xc[:], x_sb[:, img, cbk, :], AF.Abs,
                                         bias=shift[:, cbk:cbk + 1],
                                         scale=scale[:, cbk:cbk + 1])
                    nc.vector.tensor_scalar_min(xc[:], xc[:], 1.0)
                    xcs.append(xc)
                # channel sums via ones-stationary matmuls (broadcast result)
                g = gridp.tile([128, GFLAT], bf16, tag="g")
                nc.vector.memset(g[:], 0.0)
                for rt in range(NT):
                    sl = slice(rt * NPIX, (rt + 1) * NPIX)
                    pc = ps_c.tile([128, NPIX], f32)
                    nc.tensor.matmul(pc[:], ones_st[:], xcs[0][:, sl],
                                     start=True, stop=False)
                    nc.tensor.matmul(pc[:], ones_st[:], xcs[1][:, sl],
                                     start=False, stop=True)
                    gv = gview(g, 1 + (rt * RT_ROWS + 1) * PW + 1,
                               [[PW, RT_ROWS], [1, W]])
                    nc.scalar.copy(gv, pc[:].rearrange("p (h w) -> p h w",
                                                       w=W))
                bmap = box(g, boxp)
                nc.vector.scalar_tensor_tensor(
                    bbc[:, img, :],
                    gview(bmap, 1 + PW + 1, [[PW, H], [1, W]]),
                    bb_bc[:], invden[:], ALU.add, ALU.mult)

            # ---------------- 11. main conv ---------------------------------
            ov = out_d.rearrange("n (cob p) h w -> n cob p (h w)", p=128)
            for rt in range(NT):
                for cob in range(COB):
                    pys = [ps_y.tile([128, FD_CONV], f32, tag=f"py{img}",
                                     name=f"py{img}_{rt}_{cob}")
                           for img in range(NLOC)]
                    first = True
                    for cbk in range(CB):
                        for ky in range(KS):
                            for kx in range(KS):
                                off = (RT_ROWS * rt + ky) * PW + kx
                                last = (cbk == CB - 1 and ky == KS - 1
                                        and kx == KS - 1)
                                for img in range(NLOC):
                                    nc.tensor.matmul(
                                        pys[img][:],
                                        wT[:, cbk, ky * KS + kx, cob, :],
                                        t_t[cbk][img][:, off:off + FD_CONV],
                                        start=first, stop=last)
                                first = False
                    for img in range(NLOC):
                        osb = outp.tile([128, NPIX], f32, tag="osb")
                        pv = pys[img][:, 0:1]
                        pyv = bass.AP(tensor=pv.tensor, offset=pv.offset + 1,
                                      ap=[list(pv.ap[0]), [PW, RT_ROWS],
                                          [1, W]])
                        nc.vector.scalar_tensor_tensor(
                            osb[:], pyv, convb_cols[:, cob:cob + 1],
                            bbc[:, img, rt * NPIX:(rt + 1) * NPIX],
                            ALU.add, ALU.mult)
                        nc.sync.dma_start(
                            out=ov[img, cob][:, rt * NPIX:(rt + 1) * NPIX],
                            in_=osb[:])

    nc.compile()
    return nc


def _consts():
    return np.eye(128, dtype=np.float32)


def kernel(**inputs):
    from concourse.bass_utils import run_bass_kernel_spmd

    if "nc" not in _CACHE:
        _CACHE["nc"] = _build()
    nc = _CACHE["nc"]

    x = np.ascontiguousarray(inputs["x"], dtype=np.float32)
    shared = {
        "bn_gamma": np.ascontiguousarray(inputs["bn_gamma"], np.float32),
        "bn_beta": np.ascontiguousarray(inputs["bn_beta"], np.float32),
        "conv_w": np.ascontiguousarray(inputs["conv_w"], np.float32),
        "conv_b": np.ascontiguousarray(inputs["conv_b"], np.float32),
        "beta_conv_b": np.ascontiguousarray(inputs["beta_conv_b"], np.float32),
        "ident128": _consts(),
    }
    in_maps = [
        {"x": np.ascontiguousarray(x[i * NLOC:(i + 1) * NLOC]), **shared}
        for i in range(N_CORES)
    ]
    res = run_bass_kernel_spmd(nc, in_maps, list(range(N_CORES)))
    out = np.concatenate([res.results[i]["out"] for i in range(N_CORES)],
                         axis=0)
    return out.astype(np.float32)


# revision 21
# speedup vs baseline: 1.1057x; 1.1057x over previous
"""Trainium2 Bass kernel for nn_Conv2dTB (BN -> ternary quantize -> 3x3 conv
-> beta box-filter scaling), data-parallel over batch on 8 NeuronCores.

Contract: kernel(**inputs) takes the FULL unsharded inputs as numpy arrays and
returns the FULL [16, 256, 56, 56] float32 output. Internally the batch dim is
split 2 images/core; BN batch statistics use an on-device AllReduce so the
normalization matches the reference's full-batch statistics.

Phase structure (per core):
  1. dummy 4-byte AllReduce at t=0 absorbs inter-core launch skew so the real
     stats AllReduce later sees aligned peers.
  2. x streams in as 16 quarter-chunks on HWDGE; per-chunk channel sums run on
     GpSimd (reduce_sum) and channel sum-of-squares on Scalar (Square+accum),
     so stats lag the DMA by ~1us.
  3. stats AllReduce; its latency window is filled with weight bf16 conversion
     (GpSimd), PE transposes of the conv weights, and the beta-denominator
     box filter of a ones-grid (DVE).
  4. post-AR: ternarize (Scalar Sign) into flat zero-padded [58x58+2] grids,
     then the conv runs as fully contiguous 464-wide rhs slices (junk columns
     at the row seams are dropped by the output read view).
  5. beta path: channel sums via ones-stationary matmuls (output is broadcast
     across partitions), 3x3 box filter as +-1/+-58 flat shifted adds on DVE.
"""

import numpy as np

# Problem shapes (hardcoded per contract).
N, C, H, W = 16, 256, 56, 56
COUT = 256
KS = 3
EPS = 1e-4
N_CORES = 8
NLOC = N // N_CORES  # images per core (2)
CB = C // 128  # channel blocks (2)
COB = COUT // 128  # cout blocks (2)
RT_ROWS = 8  # image rows per pixel tile
NT = H // RT_ROWS  # row tiles per image (7)
NPIX = RT_ROWS * W  # valid pixels per tile (448)
HW = H * W  # 3136
PH = H + 2  # padded rows (58)
PW = W + 2  # padded cols (58)
GFLAT = PH * PW + 2  # flat padded grid: 1 front pad + 58*58 + 1 tail pad
FD_CONV = RT_ROWS * PW  # matmul free size incl junk cols (464)
QUARTER = HW // 4  # stats chunk (784)
COUNT = float(N * H * W)  # BN reduction count (full batch)

_CACHE = {}


def _build():
    import concourse.tile as tile
    import concourse.bass as bass
    from concourse import bacc, mybir

    f32 = mybir.dt.float32
    bf16 = mybir.dt.bfloat16
    AF = mybir.ActivationFunctionType
    ALU = mybir.AluOpType
    AX = mybir.AxisListType

    nc = bacc.Bacc("TRN2", target_bir_lowering=False, debug=False,
                   num_devices=N_CORES)

    # ---- external I/O ----
    x_d = nc.dram_tensor("x", [NLOC, C, H, W], f32, kind="ExternalInput").ap()
    gamma_d = nc.dram_tensor("bn_gamma", [C], f32, kind="ExternalInput").ap()
    bnbeta_d = nc.dram_tensor("bn_beta", [C], f32, kind="ExternalInput").ap()
    w_d = nc.dram_tensor("conv_w", [COUT, C, KS, KS], f32,
                         kind="ExternalInput").ap()
    cb_d = nc.dram_tensor("conv_b", [COUT], f32, kind="ExternalInput").ap()
    bb_d = nc.dram_tensor("beta_conv_b", [1], f32, kind="ExternalInput").ap()
    ident_d = nc.dram_tensor("ident128", [128, 128], f32,
                             kind="ExternalInput").ap()
    invden_d = nc.dram_tensor("invden", [H * W], f32,
                              kind="ExternalInput").ap()
    out_d = nc.dram_tensor("out", [NLOC, COUT, H, W], f32,
                           kind="ExternalOutput").ap()

    def gview(t, off, dims):
        """Strided view into a flat [128, GFLAT] tile at element offset."""
        a = t[:, 0:1]
        return bass.AP(tensor=a.tensor, offset=a.offset + off,
                       ap=[list(a.ap[0])] + [list(d) for d in dims])

    with tile.TileContext(nc) as tc:
        with (
            nc.allow_low_precision(
                reason="bf16 box-filter/beta path; error budget 2e-2"),
            tc.tile_pool(name="persist", bufs=1) as persist,
            tc.tile_pool(name="stage", bufs=4) as stage,
            tc.tile_pool(name="sq", bufs=2) as sqp,
            tc.tile_pool(name="denp", bufs=1) as denp,
            tc.tile_pool(name="xc", bufs=1) as xcp,
            tc.tile_pool(name="grid", bufs=2) as gridp,
            tc.tile_pool(name="boxt", bufs=1) as boxp,
            tc.tile_pool(name="outp", bufs=4) as outp,
            tc.tile_pool(name="ps_y", bufs=2, space="PSUM") as ps_y,
            tc.tile_pool(name="ps_c", bufs=2, space="PSUM") as ps_c,
            tc.tile_pool(name="ps_m", bufs=2, space="PSUM") as ps_m,
            tc.tile_pool(name="dram", bufs=1, space="DRAM") as dram,
        ):
            # ---------------- 1. tiny consts (sync queue, before x) ---------
            gamma_sb = persist.tile([128, CB], f32)
            nc.sync.dma_start(out=gamma_sb[:],
                              in_=gamma_d.rearrange("(cb p) -> p cb", p=128))
            bnbeta_sb = persist.tile([128, CB], f32)
            nc.sync.dma_start(out=bnbeta_sb[:],
                              in_=bnbeta_d.rearrange("(cb p) -> p cb", p=128))
            ident_sb = persist.tile([128, 128], f32)
            nc.sync.dma_start(out=ident_sb[:], in_=ident_d[:])

            # ---------------- 2. weight load on SWDGE (parallel queue) ------
            w_sb = persist.tile([128, COB, C, KS * KS], f32)
            wv = w_d.rearrange("(cob p) c k1 k2 -> cob p c (k1 k2)", p=128)
            for cob in range(COB):
                for a in range(2):
                    sl = slice(a * (C // 2), (a + 1) * (C // 2))
                    nc.gpsimd.dma_start(out=w_sb[:, cob, sl, :],
                                        in_=wv[cob][:, sl, :])

            # ---------------- 3. x stream + stats (sum on GpSimd, sumsq on
            # Scalar), chunked so stats trail the DMA by ~1 chunk ------------
            x_sb = persist.tile([128, NLOC, CB, HW], f32)
            gsum = persist.tile([128, CB, NLOC * 4], f32)
            ssq = persist.tile([128, CB, NLOC * 4], f32)
            for img in range(NLOC):
                xv = x_d[img].rearrange("(cb p) h w -> cb p (h w)", p=128)
                for cbk in range(CB):
                    for q in range(4):
                        sl = slice(q * QUARTER, (q + 1) * QUARTER)
                        nc.sync.dma_start(out=x_sb[:, img, cbk, sl],
                                          in_=xv[cbk][:, sl])
                        k = img * 4 + q
                        nc.vector.reduce_sum(gsum[:, cbk, k:k + 1],
                                             x_sb[:, img, cbk, sl], axis=AX.X)
                        sq = sqp.tile([128, QUARTER], bf16, tag="sq")
                        nc.scalar.activation(sq[:], x_sb[:, img, cbk, sl],
                                             AF.Square,
                                             accum_out=ssq[:, cbk, k:k + 1])

            # ---------------- 4. combine + AllReduce ------------------------
            partial = persist.tile([128, 2 * CB], f32)
            for cbk in range(CB):
                nc.vector.reduce_sum(partial[:, 2 * cbk:2 * cbk + 1],
                                     gsum[:, cbk, :], axis=AX.X)
                nc.vector.reduce_sum(partial[:, 2 * cbk + 1:2 * cbk + 2],
                                     ssq[:, cbk, :], axis=AX.X)
            bounce_in = dram.tile([128, 2 * CB], f32)
            bounce_out = dram.tile([128, 2 * CB], f32)
            nc.sync.dma_start(out=bounce_in[:], in_=partial[:])
            nc.gpsimd.collective_compute(
                "AllReduce", ALU.add,
                replica_groups=[list(range(N_CORES))],
                ins=[bounce_in.opt()], outs=[bounce_out.opt()],
            )
            allred = persist.tile([128, 2 * CB], f32)
            nc.sync.dma_start(out=allred[:], in_=bounce_out[:])

            # ---------------- 5. late consts --------------------------------
            convb_cols = persist.tile([128, COB], f32)
            nc.sync.dma_start(out=convb_cols[:],
                              in_=cb_d.rearrange("(cob p) -> p cob", p=128))
            bb_bc = persist.tile([128, 1], f32)
            bbsrc = bb_d[0:1]
            nc.sync.dma_start(
                out=bb_bc[:],
                in_=bass.AP(tensor=bbsrc.tensor, offset=bbsrc.offset,
                            ap=[[0, 128], [1, 1]]),
            )
            # host-computed 1/(256*boxcount+bb), broadcast to all partitions
            invden_f = denp.tile([128, HW], f32, tag="invdenf")
            iv = invden_d[0:1]
            nc.sync.dma_start(
                out=invden_f[:],
                in_=bass.AP(tensor=iv.tensor, offset=iv.offset,
                            ap=[[0, 128], [1, HW]]),
            )
            invden = persist.tile([128, HW], bf16)
            nc.vector.tensor_copy(invden[:], invden_f[:])

            # ---------------- 6. DVE prep: ident bf16, t grids, beta denom --
            ident_bf = persist.tile([128, 128], bf16)
            nc.vector.tensor_copy(ident_bf[:], ident_sb[:])
            ones_st = persist.tile([128, 128], bf16)
            nc.vector.memset(ones_st[:], 1.0)

            # ternary grids: flat [1 pad][58x58][1 pad], zeroed
            t_t = [[persist.tile([128, GFLAT], bf16, name=f"t{cbk}{img}")
                    for img in range(NLOC)] for cbk in range(CB)]
            for cbk in range(CB):
                for img in range(NLOC):
                    nc.vector.memset(t_t[cbk][img][:], 0.0)

            def box(g, rpool):
                # horizontal 3-tap into r2, then vertical 3-tap written back
                # in place over g (g is dead once r2 is computed)
                r2 = rpool.tile([128, GFLAT], bf16, tag="r2")
                nc.vector.tensor_add(r2[:, 1:GFLAT - 1], g[:, 0:GFLAT - 2],
                                     g[:, 1:GFLAT - 1])
                nc.vector.tensor_add(r2[:, 1:GFLAT - 1], r2[:, 1:GFLAT - 1],
                                     g[:, 2:GFLAT])
                lo = 1 + PW
                hi = 1 + (PH - 1) * PW
                nc.vector.tensor_add(g[:, lo:hi], r2[:, lo - PW:hi - PW],
                                     r2[:, lo:hi])
                nc.vector.tensor_add(g[:, lo:hi], g[:, lo:hi],
                                     r2[:, lo + PW:hi + PW])
                return g

            # ---------------- 7. weight prep (bf16 + PE transpose) ----------
            w_bf = persist.tile([128, COB, C, KS * KS], bf16)
            for cob in range(COB):
                nc.vector.tensor_copy(w_bf[:, cob, :, :], w_sb[:, cob, :, :])
            wT = persist.tile([128, CB, KS * KS, COB, 128], bf16)
            for cob in range(COB):
                for cbk in range(CB):
                    for tap in range(KS * KS):
                        wsl = w_bf[:, cob, cbk * 128:(cbk + 1) * 128, tap]
                        ps_t = ps_m.tile([128, 128], bf16, tag="psm")
                        nc.tensor.transpose(ps_t[:], wsl, ident_bf[:])
                        nc.scalar.copy(wT[:, cbk, tap, cob, :], ps_t[:])

            # ---------------- 8. scale/shift from AllReduce result ----------
            scale = persist.tile([128, CB], f32)
            shift = persist.tile([128, CB], f32)
            for cbk in range(CB):
                mean = stage.tile([128, 1], f32, tag="mean")
                nc.vector.tensor_scalar_mul(mean[:],
                                            allred[:, 2 * cbk:2 * cbk + 1],
                                            1.0 / COUNT)
                var = stage.tile([128, 1], f32, tag="var")
                nc.vector.tensor_mul(var[:], mean[:], mean[:])
                ex2 = stage.tile([128, 1], f32, tag="ex2")
                nc.vector.tensor_scalar_mul(
                    ex2[:], allred[:, 2 * cbk + 1:2 * cbk + 2], 1.0 / COUNT)
                nc.vector.tensor_sub(var[:], ex2[:], var[:])
                nc.vector.tensor_scalar_add(var[:], var[:], EPS)
                rvar = stage.tile([128, 1], f32, tag="rvar")
                nc.vector.reciprocal(rvar[:], var[:])
                rstd = stage.tile([128, 1], f32, tag="rstd")
                nc.scalar.sqrt(rstd[:], rvar[:])
                nc.vector.tensor_mul(scale[:, cbk:cbk + 1], rstd[:],
                                     gamma_sb[:, cbk:cbk + 1])
                nc.vector.tensor_mul(shift[:, cbk:cbk + 1], mean[:],
                                     scale[:, cbk:cbk + 1])
                nc.vector.tensor_sub(shift[:, cbk:cbk + 1],
                                     bnbeta_sb[:, cbk:cbk + 1],
                                     shift[:, cbk:cbk + 1])

            # ---------------- 9. ternarize (signs gate the conv) ------------
            for img in range(NLOC):
                for cbk in range(CB):
                    tv = gview(t_t[cbk][img], 1 + PW + 1, [[PW, H], [1, W]])
                    nc.scalar.activation(
                        tv, x_sb[:, img, cbk, :].rearrange(
                            "p (h w) -> p h w", w=W),
                        AF.Sign, bias=shift[:, cbk:cbk + 1],
                        scale=scale[:, cbk:cbk + 1])

            # ---------------- 10. beta path per image -----------------------
            bbc = persist.tile([128, NLOC, HW], bf16)
            for img in range(NLOC):
                xcs = []
                for cbk in range(CB):
                    xc = xcp.tile([128, HW], bf16, tag=f"xc{cbk}")
                    nc.scalar.activation(xc[:], x_sb[:, img, cbk, :], AF.Abs,
                                         bias=shift[:, cbk:cbk + 1],
                                         scale=scale[:, cbk:cbk + 1])
                    nc.vector.tensor_scalar_min(xc[:], xc[:], 1.0)
                    xcs.append(xc)
                # channel sums via ones-stationary matmuls (broadcast result)
                g = gridp.tile([128, GFLAT], bf16, tag="g")
                nc.vector.memset(g[:], 0.0)
                for rt in range(NT):
                    sl = slice(rt * NPIX, (rt + 1) * NPIX)
                    pc = ps_c.tile([128, NPIX], f32)
                    nc.tensor.matmul(pc[:], ones_st[:], xcs[0][:, sl],
                                     start=True, stop=False)
                    nc.tensor.matmul(pc[:], ones_st[:], xcs[1][:, sl],
                                     start=False, stop=True)
                    gv = gview(g, 1 + (rt * RT_ROWS + 1) * PW + 1,
                               [[PW, RT_ROWS], [1, W]])
                    nc.scalar.copy(gv, pc[:].rearrange("p (h w) -> p h w",
                                                       w=W))
                bmap = box(g, boxp)
                nc.vector.scalar_tensor_tensor(
                    bbc[:, img, :],
                    gview(bmap, 1 + PW + 1, [[PW, H], [1, W]]),
                    bb_bc[:], invden[:], ALU.add, ALU.mult)

            # ---------------- 11. main conv ---------------------------------
            ov = out_d.rearrange("n (cob p) h w -> n cob p (h w)", p=128)
            for rt in range(NT):
                for cob in range(COB):
                    pys = [ps_y.tile([128, FD_CONV], f32, tag=f"py{img}",
                                     name=f"py{img}_{rt}_{cob}")
                           for img in range(NLOC)]
                    first = True
                    for cbk in range(CB):
                        for ky in range(KS):
                            for kx in range(KS):
                                off = (RT_ROWS * rt + ky) * PW + kx
                                last = (cbk == CB - 1 and ky == KS - 1
                                        and kx == KS - 1)
                                for img in range(NLOC):
                                    nc.tensor.matmul(
                                        pys[img][:],
                                        wT[:, cbk, ky * KS + kx, cob, :],
                                        t_t[cbk][img][:, off:off + FD_CONV],
                                        start=first, stop=last)
                                first = False
                    for img in range(NLOC):
                        osb = outp.tile([128, NPIX], f32, tag="osb")
                        pv = pys[img][:, 0:1]
                        pyv = bass.AP(tensor=pv.tensor, offset=pv.offset + 1,
                                      ap=[list(pv.ap[0]), [PW, RT_ROWS],
                                          [1, W]])
                        nc.vector.scalar_tensor_tensor(
                            osb[:], pyv, convb_cols[:, cob:cob + 1],
                            bbc[:, img, rt * NPIX:(rt + 1) * NPIX],
                            ALU.add, ALU.mult)
                        nc.sync.dma_start(
                            out=ov[img, cob][:, rt * NPIX:(rt + 1) * NPIX],
                            in_=osb[:])

    nc.compile()
    return nc


def _consts():
    return np.eye(128, dtype=np.float32)


def _invden(bb):
    """Host-side 1/(256*boxcount + beta_conv_b) map, flat [H*W] f32."""
    r = np.minimum(np.arange(H), H - 1 - np.arange(H))
    edge = (r >= 1).astype(np.float32) + 2.0  # 2 on border rows, 3 inside
    cnt = np.outer(edge, edge).astype(np.float32)  # valid taps: 4/6/9
    return (1.0 / (256.0 * cnt + float(bb))).astype(np.float32).ravel()


def kernel(**inputs):
    from concourse.bass_utils import run_bass_kernel_spmd

    if "nc" not in _CACHE:
        _CACHE["nc"] = _build()
    nc = _CACHE["nc"]

    x = np.ascontiguousarray(inputs["x"], dtype=np.float32)
    shared = {
        "bn_gamma": np.ascontiguousarray(inputs["bn_gamma"], np.float32),
        "bn_beta": np.ascontiguousarray(inputs["bn_beta"], np.float32),
        "conv_w": np.ascontiguousarray(inputs["conv_w"], np.float32),
        "conv_b": np.ascontiguousarray(inputs["conv_b"], np.float32),
        "beta_conv_b": np.ascontiguousarray(inputs["beta_conv_b"], np.float32),
        "ident128": _consts(),
        "invden": _invden(np.asarray(inputs["beta_conv_b"]).ravel()[0]),
    }
    in_maps = [
        {"x": np.ascontiguousarray(x[i * NLOC:(i + 1) * NLOC]), **shared}
        for i in range(N_CORES)
    ]
    res = run_bass_kernel_spmd(nc, in_maps, list(range(N_CORES)))
    out = np.concatenate([res.results[i]["out"] for i in range(N_CORES)],
                         axis=0)
    return out.astype(np.float32)
